# revision 9
# baseline (speedup 1.0000x reference)
"""Trainium2 Bass kernel for 16-head causal MultiHeadAttention.

Problem: B=2, S=2048, D=1024, H=16 heads of 64. Causal mask, softmax,
fp32 weights/activations.

Sharding: tensor-parallel over heads. Each of the 8 cores handles 2 heads
(a 128-wide feature slice): it computes Q/K/V projections for its slice,
causal attention for its 2 heads over both batch elements, and a partial
output projection y_c = A_c @ Wo[c*128:(c+1)*128, :]. The host sums the 8
partials and adds bo (the "unshard" step).

Device layout notes (everything transposed, feature-on-partition):
  xT   [128, 8, 4096]   xT[p, kc, t] = x[t, kc*128+p]
  Q^T  [128, 4096]      rows = 2 heads x 64 feats, cols = token (b*2048+s)
  K^T  same
  V    [128, 32, 130]   normal layout: partition = token-within-tile,
                        per token tile: [V_h0(64) | ones | V_h1(64) | ones]
                        (ones column turns the PV matmul into a fused
                        attn+rowsum computation)
  S^T  [keys, queries]  per (b, head, 512-query chunk), computed per
                        128-key tile; exp (scale=1/8, no max subtraction:
                        scores are ~N(0,1) so exp never overflows; masked
                        entries are multiplied by 0 afterwards, matching
                        the reference's -10000 masking whose exp
                        underflows to 0 in fp32)
"""

import os
import sys
from contextlib import ExitStack

import numpy as np

for _p in ("/opt/trn_rl_repo",):
    if _p not in sys.path and os.path.isdir(_p):
        sys.path.insert(0, _p)

import concourse.bass as bass
import concourse.bacc as bacc
import concourse.tile as tile
from concourse import mybir
from concourse.bass import ts
from concourse.bass_utils import run_bass_kernel_spmd
from concourse.masks import make_identity

F32 = mybir.dt.float32
AF = mybir.ActivationFunctionType

B, S, D, H, HD = 2, 2048, 1024, 16, 64
T = B * S                     # 4096 tokens
NCORES = 8
FPC = D // NCORES             # 128 features per core (2 heads)
HPC = FPC // HD               # 2 heads per core
KC = D // 128                 # 8 contraction chunks for projections
TCH = T // 512                # 8 token chunks of 512
QCH = S // 512                # 4 query chunks per batch
NTT = T // 128                # 32 token tiles of 128

USE_F32R = os.environ.get("MHA_F32R", "1") == "1"
DEBUG_DUMP = os.environ.get("MHA_DEBUG", "0") == "1"


def build_nc(use_f32r: bool = USE_F32R, debug: bool = DEBUG_DUMP) -> bass.Bass:
    nc = bacc.Bacc()

    MM = mybir.dt.float32r if use_f32r else F32
    xT = nc.declare_dram_parameter("xT", [128, KC, T], MM, False)
    wq = nc.declare_dram_parameter("wq", [128, KC, FPC], MM, False)
    wk = nc.declare_dram_parameter("wk", [128, KC, FPC], MM, False)
    wv = nc.declare_dram_parameter("wv", [128, KC, FPC], MM, False)
    wo = nc.declare_dram_parameter("wo", [FPC, D], MM, False)
    bq = nc.declare_dram_parameter("bq", [FPC, 1], F32, False)
    bk = nc.declare_dram_parameter("bk", [FPC, 1], F32, False)
    bv = nc.declare_dram_parameter("bv", [FPC, 1], F32, False)
    maskT = nc.declare_dram_parameter("maskT", [128, 4, 512], MM, False)
    vones = nc.declare_dram_parameter("vones", [128, NTT, 2, 1], MM, False)
    yT = nc.declare_dram_parameter("yT", [D, T], F32, True)
    if debug:
        MMd = mybir.dt.float32r if use_f32r else F32
        dbg_qt = nc.declare_dram_parameter("dbg_qt", [128, T], MMd, True)
        dbg_kt = nc.declare_dram_parameter("dbg_kt", [128, T], MMd, True)
        dbg_v = nc.declare_dram_parameter("dbg_v", [128, NTT, 2 * (HD + 1)], MMd, True)
        dbg_at = nc.declare_dram_parameter("dbg_at", [128, T], MMd, True)
        dbg_acc = nc.declare_dram_parameter("dbg_acc", [65, 512], F32, True)
        dbg_rmat = nc.declare_dram_parameter("dbg_rmat", [64, 512], F32, True)

    with tile.TileContext(nc) as tc, ExitStack() as ctx:
        const = ctx.enter_context(tc.tile_pool(name="const", bufs=1))
        persist = ctx.enter_context(tc.tile_pool(name="persist", bufs=1))
        xt_pool = ctx.enter_context(tc.tile_pool(name="xt_pool", bufs=2))
        vt_pool = ctx.enter_context(tc.tile_pool(name="vt_pool", bufs=2))
        pt_pool = ctx.enter_context(tc.tile_pool(name="pt_pool", bufs=3))
        yt_pool = ctx.enter_context(tc.tile_pool(name="yt_pool", bufs=3))
        rmat_pool = ctx.enter_context(tc.tile_pool(name="rmat_pool", bufs=2))
        recip_pool = ctx.enter_context(tc.tile_pool(name="recip_pool", bufs=2))
        tmp_pool = ctx.enter_context(tc.tile_pool(name="tmp_pool", bufs=2))

        wq_sb = const.tile([128, KC, FPC], MM)
        wk_sb = const.tile([128, KC, FPC], MM)
        wv_sb = const.tile([128, KC, FPC], MM)
        wo_sb = const.tile([FPC, D], MM)
        bq_sb = const.tile([FPC, 1], F32)
        bk_sb = const.tile([FPC, 1], F32)
        bv_sb = const.tile([FPC, 1], F32)
        mask_sb = const.tile([128, 4, 512], MM)
        ident = const.tile([128, 128], F32)
        ones65 = const.tile([65, 64], F32)
        nc.sync.dma_start(out=wq_sb, in_=wq[:])
        nc.sync.dma_start(out=wk_sb, in_=wk[:])
        nc.sync.dma_start(out=wv_sb, in_=wv[:])
        nc.sync.dma_start(out=wo_sb, in_=wo[:])
        nc.sync.dma_start(out=bq_sb, in_=bq[:])
        nc.sync.dma_start(out=bk_sb, in_=bk[:])
        nc.sync.dma_start(out=bv_sb, in_=bv[:])
        nc.sync.dma_start(out=mask_sb, in_=maskT[:])
        make_identity(nc, ident)
        nc.vector.memset(ones65, 1.0)

        QT = persist.tile([128, T], MM)
        KT = persist.tile([128, T], MM)
        V = persist.tile([128, NTT, 2 * (HD + 1)], MM)
        AT = persist.tile([128, T], MM)
        vslots = V.rearrange("p t (g x) -> p t g x", g=2)  # x = 65
        nc.sync.dma_start(out=vslots[:, :, :, HD : HD + 1], in_=vones[:])

        # ---- projections: Q^T, K^T (feature-major) and V (token-major) ----
        with (
            tc.tile_pool(name="proj_ps", bufs=4, space="PSUM") as proj_ps,
            tc.tile_pool(name="tr_ps", bufs=2, space="PSUM") as tr_ps,
        ):
            for tcn in range(TCH):
                xt = xt_pool.tile([128, KC, 512], MM)
                nc.sync.dma_start(out=xt, in_=xT[:, :, ts(tcn, 512)])
                for wsb, bsb, dest in (
                    (wq_sb, bq_sb, QT),
                    (wk_sb, bk_sb, KT),
                ):
                    ps = proj_ps.tile([128, 512], F32, name="proj_psum")
                    for kc in range(KC):
                        nc.tensor.matmul(
                            ps,
                            wsb[:, kc, :],
                            xt[:, kc, :],
                            start=(kc == 0),
                            stop=(kc == KC - 1),
                        )
                    nc.scalar.activation(
                        dest[:, ts(tcn, 512)], ps, AF.Identity, bias=bsb
                    )
                # V^T chunk then transpose into token-major V
                ps = proj_ps.tile([128, 512], F32, name="proj_psum")
                for kc in range(KC):
                    nc.tensor.matmul(
                        ps,
                        wv_sb[:, kc, :],
                        xt[:, kc, :],
                        start=(kc == 0),
                        stop=(kc == KC - 1),
                    )
                vt = vt_pool.tile([128, 512], F32)
                nc.scalar.activation(vt, ps, AF.Identity, bias=bv_sb)
                for i in range(4):
                    tp = tr_ps.tile([128, 128], F32, name="tr_psum")
                    nc.tensor.transpose(tp, vt[:, ts(i, 128)], ident)
                    tt = tcn * 4 + i
                    nc.vector.tensor_copy(
                        vslots[:, tt, :, 0:HD],
                        tp.rearrange("p (g f) -> p g f", g=2),
                    )

        # ---- attention + interleaved output projection ----
        with (
            tc.tile_pool(name="st_ps", bufs=2, space="PSUM") as st_ps,
            tc.tile_pool(name="acc_ps", bufs=2, space="PSUM") as acc_ps,
            tc.tile_pool(name="op_ps", bufs=2, space="PSUM") as op_ps,
        ):
            for b in range(B):
                for qc in range(QCH):
                    g0 = b * S + qc * 512
                    for hl in range(HPC):
                        hb = hl * HD
                        nkt = 4 * (qc + 1)  # 128-key tiles (causal)
                        acc = acc_ps.tile([HD + 1, 512], F32, name="acc_psum")
                        for j in range(nkt // 2):
                            st = st_ps.tile([128, 1024], F32, name="st_psum")
                            for half in range(2):
                                kt = 2 * j + half
                                k0 = b * S + kt * 128
                                nc.tensor.matmul(
                                    st[:, ts(half, 512)],
                                    KT[hb : hb + HD, k0 : k0 + 128],
                                    QT[hb : hb + HD, g0 : g0 + 512],
                                    start=True,
                                    stop=True,
                                )
                            pt = pt_pool.tile([128, 1024], MM)
                            nc.scalar.activation(pt, st, AF.Exp, scale=0.125)
                            for half in range(2):
                                kt = 2 * j + half
                                d = kt - 4 * qc
                                if d >= 0:
                                    nc.vector.tensor_mul(
                                        pt[:, ts(half, 512)],
                                        pt[:, ts(half, 512)],
                                        mask_sb[:, d, :],
                                    )
                            for half in range(2):
                                kt = 2 * j + half
                                nc.tensor.matmul(
                                    acc,
                                    vslots[:, b * (S // 128) + kt, hl, :],
                                    pt[:, ts(half, 512)],
                                    start=(kt == 0),
                                    stop=(kt == nkt - 1),
                                )
                        recip = recip_pool.tile([HD + 1, 512], F32)
                        nc.vector.reciprocal(
                            recip[HD : HD + 1, :], acc[HD : HD + 1, :]
                        )
                        rmat_ps = op_ps.tile([128, 512], F32, name="op_psum")
                        nc.tensor.matmul(
                            rmat_ps[0:HD, :],
                            ones65[HD : HD + 1, :],
                            recip[HD : HD + 1, :],
                            start=True,
                            stop=True,
                        )
                        rmat = rmat_pool.tile([HD, 512], F32)
                        nc.scalar.activation(rmat, rmat_ps[0:HD, :], AF.Copy)
                        if debug and b == 0 and qc == 0 and hl == 0:
                            acc_sb = tmp_pool.tile([65, 512], F32, name="acc_sb")
                            nc.scalar.activation(acc_sb, acc, AF.Copy)
                            nc.sync.dma_start(out=dbg_acc[:], in_=acc_sb)
                            nc.sync.dma_start(out=dbg_rmat[:], in_=rmat)
                        if hl == 0:
                            nc.vector.tensor_mul(
                                AT[0:HD, g0 : g0 + 512], acc[0:HD, :], rmat
                            )
                        else:
                            tmp = tmp_pool.tile([HD, 512], F32)
                            nc.vector.tensor_mul(tmp, acc[0:HD, :], rmat)
                            sh = op_ps.tile([128, 512], F32, name="op_psum")
                            nc.tensor.matmul(
                                sh[HD : 2 * HD, :],
                                ident[0:HD, 0:HD],
                                tmp,
                                start=True,
                                stop=True,
                            )
                            nc.scalar.activation(
                                AT[HD : 2 * HD, g0 : g0 + 512],
                                sh[HD : 2 * HD, :],
                                AF.Copy,
                            )
                    # partial output projection for this 512-token chunk
                    for mt in range(D // 128):
                        ps = op_ps.tile([128, 512], F32, name="op_psum")
                        nc.tensor.matmul(
                            ps,
                            wo_sb[:, ts(mt, 128)],
                            AT[:, g0 : g0 + 512],
                            start=True,
                            stop=True,
                        )
                        yt = yt_pool.tile([128, 512], F32)
                        nc.vector.tensor_copy(yt, ps)
                        nc.sync.dma_start(
                            out=yT[ts(mt, 128), g0 : g0 + 512], in_=yt
                        )
        if debug:
            nc.sync.dma_start(out=dbg_qt[:], in_=QT)
            nc.sync.dma_start(out=dbg_kt[:], in_=KT)
            nc.sync.dma_start(out=dbg_v[:], in_=V)
            nc.sync.dma_start(out=dbg_at[:], in_=AT)

    nc.finalize()
    return nc


def _install_ntff_hook():
    """bass_utils' trace path needs antenv.axon_hooks, which this image's
    antenv lacks; synthesize it from the boot helper so NTFF profiling works."""
    try:
        from antenv.axon_hooks import get_axon_ntff_profile_hook  # noqa: F401

        return
    except ImportError:
        pass
    try:
        import types

        import antenv
        from trn_agent_boot.trn_boot import _ntff_profile_via_ctypes

        hook = _ntff_profile_via_ctypes("/opt/axon/libaxon_pjrt.so")
        mod = types.ModuleType("antenv.axon_hooks")
        state = {"hook": hook}
        mod.get_axon_ntff_profile_hook = lambda: state["hook"]
        mod.set_axon_ntff_profile_hook = lambda h: state.update(hook=h)
        sys.modules["antenv.axon_hooks"] = mod
        antenv.axon_hooks = mod
    except Exception:
        pass


_NC_CACHE: dict[bool, bass.Bass] = {}


def _get_nc(use_f32r: bool) -> bass.Bass:
    if use_f32r not in _NC_CACHE:
        _NC_CACHE[use_f32r] = build_nc(use_f32r)
    return _NC_CACHE[use_f32r]


def _shard_inputs(inputs, Wq, bq, Wk, bk, Wv, bv, Wo, bo):
    x = np.ascontiguousarray(np.asarray(inputs, dtype=np.float32)).reshape(T, D)
    # xT[p, kc, t] = x[t, kc*128+p]
    xTh = np.ascontiguousarray(x.reshape(T, KC, 128).transpose(2, 1, 0))

    maskh = np.zeros((128, 4, 512), dtype=np.float32)
    p = np.arange(128)[:, None]
    jj = np.arange(512)[None, :]
    for d in range(4):
        maskh[:, d, :] = (d * 128 + p <= jj).astype(np.float32)

    def wslice(W, c):
        Wc = np.asarray(W, dtype=np.float32)[:, c * FPC : (c + 1) * FPC]
        # [128, KC, FPC] with [p, kc, m] = W[kc*128+p, m]
        return np.ascontiguousarray(Wc.reshape(KC, 128, FPC).transpose(1, 0, 2))

    in_maps = []
    for c in range(NCORES):
        in_maps.append(
            {
                "xT": xTh,
                "wq": wslice(Wq, c),
                "wk": wslice(Wk, c),
                "wv": wslice(Wv, c),
                "wo": np.ascontiguousarray(
                    np.asarray(Wo, dtype=np.float32)[c * FPC : (c + 1) * FPC, :]
                ),
                "bq": np.asarray(bq, np.float32)[c * FPC : (c + 1) * FPC, None],
                "bk": np.asarray(bk, np.float32)[c * FPC : (c + 1) * FPC, None],
                "bv": np.asarray(bv, np.float32)[c * FPC : (c + 1) * FPC, None],
                "maskT": maskh,
                "vones": np.ones((128, NTT, 2, 1), dtype=np.float32),
            }
        )
    return in_maps


def run_with_results(
    inputs,
    Wq,
    bq,
    Wk,
    bk,
    Wv,
    bv,
    Wo,
    bo,
    trace: bool = False,
    use_f32r: bool = USE_F32R,
):
    in_maps = _shard_inputs(inputs, Wq, bq, Wk, bk, Wv, bv, Wo, bo)
    if trace:
        _install_ntff_hook()
    nc = _get_nc(use_f32r)
    res = run_bass_kernel_spmd(
        nc, in_maps, core_ids=list(range(NCORES)), trace=trace
    )
    acc = np.zeros((D, T), dtype=np.float32)
    for c in range(NCORES):
        acc += res.results[c]["yT"]
    y = acc.T + np.asarray(bo, np.float32)[None, :]
    out = np.ascontiguousarray(y.reshape(B, S, D).astype(np.float32))
    return out, res


def kernel(**inputs) -> np.ndarray:
    out, _ = run_with_results(**inputs)
    return out


if __name__ == "__main__":
    nc = build_nc()
    print("built ok")
